# revision 1
# baseline (speedup 1.0000x reference)
"""CascadeNNBN Trainium2 kernel.

8-way data-parallel over the batch dim. Each core holds a 2048-row shard
of the batch with features kept TRANSPOSED in SBUF (features on
partitions, batch on the free axis), so every cascade matmul contracts
over the partition dim with no on-device transposes:

    h_i^T [256, 2048] = W_i @ feats^T   (lhsT = W_i^T, host-pretransposed)

BatchNorm batch statistics are raw per-core (sum, sum-of-squares) pairs,
exchanged with one tiny AllGather per stage (lower latency floor than
AllReduce) and summed locally. Only the last two k-tiles of each stage's
contraction depend on the exchanged stats (BN is a per-feature affine
transform), so each collective's latency is hidden behind AR-independent
matmuls: the next stage's ungated k-tiles, slices of the output matmul,
and "parked" partial accumulations of later stages (kept in SBUF and
re-injected into PSUM exactly via an identity matmul), scheduled into
each collective window by an explicit fill plan.

Matmuls run in bf16 (fp32 PSUM accumulation); statistics, normalization
coefficients and the final output are fp32.
"""

import sys

if "/opt/trn_rl_repo" not in sys.path:
    sys.path.insert(0, "/opt/trn_rl_repo")

import numpy as np
from ml_dtypes import bfloat16

import concourse.bass as bass  # noqa: F401  (import keeps bass registered)
import concourse.mybir as mybir
import concourse.tile as tile
from concourse import bacc
from concourse.bass_utils import run_bass_kernel_spmd
from concourse.masks import make_identity

N_CORES = 8
B = 16384
BSH = B // N_CORES          # 2048 batch rows per core
DIN = 512
K = 8                       # cascade stages
WS = 256                    # neurons per stage
DOUT = 128
EPS = 1e-5
P = 128
NB = BSH // 512             # batch chunks of 512 (PSUM bank free dim)
KO = [(DIN + WS * i) // P for i in range(K)]   # k-tiles per stage: 4,6,...,18
T_TOT = (DIN + WS * K) // P                    # 20 F tiles
HACC_STAGES = (2, 3, 4, 5, 6, 7)  # stages whose x-part is pre-accumulated

_NC_CACHE = {}

# test-harness knobs (ignored in normal use): when TRACE_DIR is set the
# device run is profiled and kernel() stores the BassKernelResults here.
TRACE_DIR = None
LAST_RESULTS = None

BF = mybir.dt.bfloat16
F32 = mybir.dt.float32


def _build_nc():
    nc = bacc.Bacc("TRN2", target_bir_lowering=False, debug=False,
                   num_devices=N_CORES)

    xt_d = nc.dram_tensor("xt", [P, DIN // P, BSH], BF, kind="ExternalInput")
    w_d = [
        nc.dram_tensor(f"w{i}", [P, KO[i], WS], BF, kind="ExternalInput")
        for i in range(K)
    ]
    wo_d = nc.dram_tensor("wo", [P, T_TOT, DOUT], BF, kind="ExternalInput")
    bv_d = nc.dram_tensor("bv", [P, K, 2], F32, kind="ExternalInput")
    gv_d = nc.dram_tensor("gv", [P, K, 2], F32, kind="ExternalInput")
    bev_d = nc.dram_tensor("bev", [P, K, 2], F32, kind="ExternalInput")
    bout_d = nc.dram_tensor("boutv", [P, 1], F32, kind="ExternalInput")
    outT_d = nc.dram_tensor("outT", [P, BSH], F32, kind="ExternalOutput")

    with tile.TileContext(nc) as tc:
        _emit(nc, tc, xt_d, w_d, wo_d, bv_d, gv_d, bev_d, bout_d, outT_d)
    nc.compile()
    return nc


def _emit(nc, tc, xt_d, w_d, wo_d, bv_d, gv_d, bev_d, bout_d, outT_d):
    AF = mybir.ActivationFunctionType
    OP = mybir.AluOpType
    groups = [list(range(N_CORES))]

    with (
        tc.tile_pool(name="big", bufs=1) as big,
        tc.tile_pool(name="hp", bufs=1) as hp,
        tc.tile_pool(name="scrp", bufs=3) as scrp,
        tc.tile_pool(name="small", bufs=2) as small,
        tc.tile_pool(name="ps", bufs=8, space="PSUM") as ps,
        tc.tile_pool(name="dram", bufs=2, space="DRAM") as dram,
    ):
        # ---- persistent SBUF ----
        F = [big.tile([P, BSH], BF, tag=f"F{t}", name=f"F{t}") for t in range(T_TOT)]
        Wsb = [big.tile([P, KO[i], WS], BF, tag=f"W{i}", name=f"W{i}") for i in range(K)]
        WO = big.tile([P, T_TOT, DOUT], BF, tag="WO")
        BV = big.tile([P, K, 2], F32, tag="BV")
        GV = big.tile([P, K, 2], F32, tag="GV")
        BEV = big.tile([P, K, 2], F32, tag="BEV")
        BOUT = big.tile([P, 1], F32, tag="BOUT")
        OUTACC = big.tile([P, BSH], F32, tag="OUTACC")
        EPSC = big.tile([P, 1], F32, tag="EPSC")
        nc.vector.memset(EPSC[:], EPS)
        # identity (bf16) for re-injecting SBUF partial sums into PSUM
        IDT = big.tile([P, P], BF, tag="IDT")
        make_identity(nc, IDT[:, :])
        # x-block partial pre-accumulation targets for stages 2..5: their
        # x-part matmuls run inside the long first-collective window and
        # are re-injected later via an (exact) identity matmul.
        HACC = {j: big.tile([P, 2, BSH], BF, tag=f"HACC{j}", name=f"HACC{j}")
                for j in HACC_STAGES}

        # ---- input DMAs: three queues, ordered by first consumption ----
        # sync: x-blocks then late-stage weights; gpsimd: stage-0 weights,
        # output weights, small vectors; scalar: early-stage weights.
        for ko in range(KO[0]):
            nc.gpsimd.dma_start(Wsb[0][:, ko, :], w_d[0][:, ko, :])
        nc.gpsimd.dma_start(WO[:], wo_d[:, :, :])
        nc.gpsimd.dma_start(BV[:], bv_d[:, :, :])
        nc.gpsimd.dma_start(GV[:], gv_d[:, :, :])
        nc.gpsimd.dma_start(BEV[:], bev_d[:, :, :])
        nc.gpsimd.dma_start(BOUT[:], bout_d[:, :])
        nc.gpsimd.dma_start(Wsb[3][:], w_d[3][:, :, :])
        nc.gpsimd.dma_start(Wsb[7][:], w_d[7][:, :, :])
        for t in range(DIN // P):
            eng = nc.sync if t % 2 == 0 else nc.scalar
            for bb in range(NB):
                eng.dma_start(F[t][:, bb * 512:(bb + 1) * 512],
                              xt_d[:, t, bb * 512:(bb + 1) * 512])
        nc.sync.dma_start(Wsb[5][:], w_d[5][:, :, :])
        nc.sync.dma_start(Wsb[6][:], w_d[6][:, :, :])
        nc.scalar.dma_start(Wsb[1][:], w_d[1][:, :, :])
        nc.scalar.dma_start(Wsb[2][:], w_d[2][:, :, :])
        nc.scalar.dma_start(Wsb[4][:], w_d[4][:, :, :])

        # stage j's k-tiles pre-accumulated into HACC[j] before its real
        # PSUM group materializes (parked in SBUF, re-injected exactly via
        # an identity matmul)
        PREACC = {2: 4, 3: 4, 4: 6, 5: 8, 6: 6, 7: 8}  # parked k-tiles

        def mm(pt, lhsT, k, bb, start, stop):
            nc.tensor.matmul(
                pt[:, :], lhsT, F[k][:, bb * 512:(bb + 1) * 512],
                start=start, stop=stop)

        def alloc_group():
            return [
                [ps.tile([P, 512], F32, tag="pt", name="pt") for _ in range(NB)]
                for _ in range(2)
            ]

        def hacc_group(j, ks, init):
            """Advance stage j's parked partial sum by k-tiles ``ks``."""
            pa = alloc_group()
            for n in range(2):
                for bb in range(NB):
                    if not init:
                        nc.tensor.matmul(
                            pa[n][bb][:, :], IDT[:, :],
                            HACC[j][:, n, bb * 512:(bb + 1) * 512],
                            start=True, stop=False)
                for k in ks:
                    lhsT = Wsb[j][:, k, n * P:(n + 1) * P]
                    for bb in range(NB):
                        mm(pa[n][bb], lhsT, k, bb,
                           init and k == ks[0], k == ks[-1])
            for n in range(2):
                for bb in range(NB):
                    nc.vector.tensor_copy(
                        HACC[j][:, n, bb * 512:(bb + 1) * 512], pa[n][bb][:, :])

        def real_early(j, psums):
            """Ungated part of stage j's real contraction."""
            pre = PREACC.get(j, 0)
            for n in range(2):
                if pre:
                    for bb in range(NB):
                        nc.tensor.matmul(
                            psums[n][bb][:, :], IDT[:, :],
                            HACC[j][:, n, bb * 512:(bb + 1) * 512],
                            start=True, stop=False)
                for k in range(pre, KO[j] - 2):
                    lhsT = Wsb[j][:, k, n * P:(n + 1) * P]
                    for bb in range(NB):
                        mm(psums[n][bb], lhsT, k, bb,
                           pre == 0 and k == 0, False)

        def real_late(j, psums):
            # chunk-major order: psum (n, bb) groups complete progressively
            # so the relu/stats pipeline starts before the last matmul
            for bb in range(NB):
                for n in range(2):
                    for k in (KO[j] - 2, KO[j] - 1):
                        lhsT = Wsb[j][:, k, n * P:(n + 1) * P]
                        mm(psums[n][bb], lhsT, k, bb, False, k == KO[j] - 1)

        def out_group(ks, first):
            pso = [ps.tile([P, 512], F32, tag="pt", name="pt")
                   for _ in range(NB)]
            for k in ks:
                lhsT = WO[:, k, :]
                for bb in range(NB):
                    nc.tensor.matmul(
                        pso[bb][:, :], lhsT,
                        F[k][:, bb * 512:(bb + 1) * 512],
                        start=(k == ks[0]), stop=(k == ks[-1]))
            for bb in range(NB):
                dst = OUTACC[:, bb * 512:(bb + 1) * 512]
                if first:
                    nc.vector.tensor_scalar_add(dst, pso[bb][:, :],
                                                BOUT[:, 0:1])
                else:
                    nc.vector.tensor_add(dst, dst, pso[bb][:, :])

        # per-window AR-independent fill: list of thunks emitted between
        # the AllGather trigger and the BN-coefficient computation of each
        # iteration, sized to cover the collective's latency
        WINDOW_FILL = {
            0: [lambda: out_group(range(4), True),
                lambda: hacc_group(2, range(4), True),
                lambda: hacc_group(3, range(4), True),
                lambda: hacc_group(4, range(4), True),
                lambda: hacc_group(5, range(4), True),
                lambda: hacc_group(7, range(4), True),
                lambda: hacc_group(6, range(4), True)],
            1: [lambda: hacc_group(4, (4, 5), False),
                lambda: hacc_group(5, (4, 5), False)],
            2: [lambda: hacc_group(5, (6, 7), False),
                lambda: hacc_group(6, (4, 5), False)],
            3: [lambda: hacc_group(7, (4, 5), False)],
            4: [lambda: hacc_group(7, (6, 7), False)],
            5: [],
            6: [],
            7: [lambda: out_group(range(4, T_TOT - 2), False)],
        }

        # stage 0: everything available immediately
        psums = alloc_group()
        real_early(0, psums)
        real_late(0, psums)

        for i in range(K):
            # ---- relu + bias: PSUM -> bf16 h in SBUF ----
            # n=0 chunks on ScalarE, n=1 chunks on VectorE so both n-tiles
            # clear PSUM (and feed bn_stats) in parallel.
            hs = [hp.tile([P, BSH], BF, tag=f"h{n}", name=f"h{n}") for n in range(2)]
            sums = small.tile([P, NB], F32, tag="sums")
            sqs = small.tile([P, NB], F32, tag="sqs")
            st = small.tile([P, NB, 6], F32, tag="st")
            mv1 = small.tile([P, 2], F32, tag="mv1")
            arin = small.tile([P, 2, 2], F32, tag="arin")
            # stats payload = raw (sum, sum-of-squares) pairs, additive
            # across cores. n=0 on ScalarE (relu/square with accum_out),
            # n=1 on VectorE (relu + bn_stats), in psum-completion order so
            # both pipelines drain right behind the last matmul.
            for bb in range(NB):
                c0 = hs[0][:, bb * 512:(bb + 1) * 512]
                c1 = hs[1][:, bb * 512:(bb + 1) * 512]
                nc.scalar.activation(
                    c0, psums[0][bb][:, :], AF.Relu,
                    bias=BV[:, i, 0:1], scale=1.0,
                    accum_out=sums[:, bb:bb + 1],
                )
                scr = scrp.tile([P, 512], BF, tag="scr", name="scr")
                nc.scalar.activation(
                    scr[:, :], c0, AF.Square,
                    accum_out=sqs[:, bb:bb + 1])
                nc.vector.tensor_scalar(
                    c1, psums[1][bb][:, :], BV[:, i, 1:2], 0.0,
                    op0=OP.add, op1=OP.max,
                )
                nc.vector.bn_stats(st[:, bb, :], c1)
            nc.vector.bn_aggr(mv1[:], st[:, :, :])
            # n=1: (mean, var) -> raw sums;  n=0: reduce the chunk sums
            nc.vector.tensor_scalar(
                arin[:, 1, 1:2], mv1[:, 0:1], mv1[:, 0:1], mv1[:, 1:2],
                op0=OP.mult, op1=OP.add)
            nc.vector.tensor_scalar_mul(arin[:, 1, 1:2], arin[:, 1, 1:2],
                                        float(BSH))
            nc.vector.tensor_scalar_mul(arin[:, 1, 0:1], mv1[:, 0:1],
                                        float(BSH))
            nc.vector.tensor_reduce(
                arin[:, 0, 0:1], sums[:, :], axis=mybir.AxisListType.X,
                op=OP.add)
            nc.vector.tensor_reduce(
                arin[:, 0, 1:2], sqs[:, :], axis=mybir.AxisListType.X,
                op=OP.add)

            # ---- cross-core exchange of the stats (2KB): AllGather has a
            # ~2x lower latency floor than AllReduce; the 8-way sum is done
            # locally on VectorE ----
            ccin = dram.tile([P, 2, 2], F32, tag="ccin")
            ccout = dram.tile([N_CORES, P, 2, 2], F32, tag="ccout",
                              addr_space="Shared")
            nc.sync.dma_start(ccin[:], arin[:])
            nc.gpsimd.collective_compute(
                "AllGather", OP.bypass, replica_groups=groups,
                ins=[ccin.opt()], outs=[ccout.opt()],
            )
            ag = small.tile([P, N_CORES, 2, 2], F32, tag="ag")
            nc.sync.dma_start(ag[:], ccout[:, :, :, :].rearrange(
                "r p a b -> p r a b"))
            ared = small.tile([P, 2, 2], F32, tag="ared")
            nc.vector.tensor_reduce(
                ared[:, :, :], ag[:, :, :, :].rearrange("p r a b -> p a b r"),
                axis=mybir.AxisListType.X, op=OP.add)

            # ---- overlap window: AR-independent matmul fill ----
            for thunk in WINDOW_FILL[i]:
                thunk()
            if i < K - 1:
                psums = alloc_group()
                real_early(i + 1, psums)

            # ---- BN affine coefficients from global stats ----
            mue = small.tile([P, 2, 2], F32, tag="mue")
            nv = small.tile([P, 2], F32, tag="nv")
            rstd = small.tile([P, 2], F32, tag="rstd")
            a_ = small.tile([P, 2], F32, tag="a_")
            negc = small.tile([P, 2], F32, tag="negc")
            nc.vector.tensor_scalar_mul(mue[:], ared[:, :, :], 1.0 / B)
            mu = mue[:, :, 0]
            nc.vector.tensor_mul(nv[:], mu, mu)
            nc.vector.tensor_sub(nv[:], nv[:], mue[:, :, 1])   # mu^2 - E2 = -var
            nc.scalar.activation(rstd[:], nv[:], AF.Sqrt,
                                 bias=EPSC[:, 0:1], scale=-1.0)  # sqrt(var+eps)
            nc.vector.reciprocal(rstd[:], rstd[:])
            nc.vector.tensor_mul(a_[:], GV[:, i, :], rstd[:])
            nc.vector.tensor_mul(negc[:], mu, a_[:])
            nc.vector.tensor_sub(negc[:], negc[:], BEV[:, i, :])  # a*mu - beta

            # ---- normalize into the F blocks (bf16), chunked so the
            # first gated matmul can start before the whole tile is done ----
            for n in range(2):
                for q in range(NB):
                    sl = slice(q * 512, (q + 1) * 512)
                    nc.vector.tensor_scalar(
                        F[DIN // P + 2 * i + n][:, sl], hs[n][:, sl],
                        a_[:, n:n + 1], negc[:, n:n + 1],
                        op0=OP.mult, op1=OP.subtract,
                    )

            # ---- gated (late) matmuls of the next stage ----
            if i < K - 1:
                real_late(i + 1, psums)

        # ---- epilogue: last two output k-tiles + store ----
        out_group((T_TOT - 2, T_TOT - 1), False)
        for bb in range(NB):
            nc.sync.dma_start(outT_d[:, bb * 512:(bb + 1) * 512],
                              OUTACC[:, bb * 512:(bb + 1) * 512])


def _get_nc():
    if "nc" not in _NC_CACHE:
        _NC_CACHE["nc"] = _build_nc()
    return _NC_CACHE["nc"]


def kernel(x, W0, W1, W2, W3, W4, W5, W6, W7, b, gamma, beta, Wout, bout):
    Ws = [W0, W1, W2, W3, W4, W5, W6, W7]
    nc = _get_nc()

    def pack_vec(v):  # [8,256] -> [128, 8, 2]
        return np.ascontiguousarray(
            np.asarray(v, np.float32).reshape(K, 2, P).transpose(2, 0, 1))

    common = {}
    for i, W in enumerate(Ws):
        wt = np.asarray(W, np.float32).T.astype(bfloat16)        # [d_i, 256]
        common[f"w{i}"] = np.ascontiguousarray(
            wt.reshape(KO[i], P, WS).transpose(1, 0, 2))         # [128, ko, 256]
    wot = np.asarray(Wout, np.float32).T.astype(bfloat16)        # [2560, 128]
    common["wo"] = np.ascontiguousarray(
        wot.reshape(T_TOT, P, DOUT).transpose(1, 0, 2))          # [128, 20, 128]
    common["bv"] = pack_vec(b)
    common["gv"] = pack_vec(gamma)
    common["bev"] = pack_vec(beta)
    common["boutv"] = np.ascontiguousarray(
        np.asarray(bout, np.float32).reshape(P, 1))

    in_maps = []
    for c in range(N_CORES):
        xs = np.asarray(x[c * BSH:(c + 1) * BSH], np.float32)    # [2048, 512]
        xt = xs.T.astype(bfloat16)                               # [512, 2048]
        in_maps.append({
            **common,
            "xt": np.ascontiguousarray(
                xt.reshape(DIN // P, P, BSH).transpose(1, 0, 2)),
        })

    kw = {}
    if TRACE_DIR is not None:
        kw = dict(trace=True, tmpdir=TRACE_DIR)
    try:
        res = run_bass_kernel_spmd(nc, in_maps, list(range(N_CORES)), **kw)
    except Exception:
        # transient PJRT INTERNAL errors have been observed; retry once
        res = run_bass_kernel_spmd(nc, in_maps, list(range(N_CORES)), **kw)
    global LAST_RESULTS
    LAST_RESULTS = res
    out = np.empty((B, DOUT), np.float32)
    for c in range(N_CORES):
        out[c * BSH:(c + 1) * BSH] = res.results[c]["outT"].T
    return out



# revision 2
# speedup vs baseline: 1.1181x; 1.1181x over previous
"""CascadeNNBN Trainium2 kernel.

8-way data-parallel over the batch dim. Each core holds a 2048-row shard
of the batch with features kept TRANSPOSED in SBUF (features on
partitions, batch on the free axis), so every cascade matmul contracts
over the partition dim with no on-device transposes:

    h_i^T [256, 2048] = W_i @ feats^T   (lhsT = W_i^T, host-pretransposed)

BatchNorm batch statistics are raw per-core (sum, sum-of-squares) pairs,
exchanged with one tiny AllGather per stage (lower latency floor than
AllReduce) and summed locally. Only the last two k-tiles of each stage's
contraction depend on the exchanged stats (BN is a per-feature affine
transform), so each collective's latency is hidden behind AR-independent
matmuls: the next stage's ungated k-tiles, slices of the output matmul,
and "parked" partial accumulations of later stages (kept in SBUF and
re-injected into PSUM exactly via an identity matmul), scheduled into
each collective window by an explicit fill plan.

Matmuls run in bf16 (fp32 PSUM accumulation); statistics, normalization
coefficients and the final output are fp32.
"""

import sys

if "/opt/trn_rl_repo" not in sys.path:
    sys.path.insert(0, "/opt/trn_rl_repo")

import numpy as np
from ml_dtypes import bfloat16

import concourse.bass as bass  # noqa: F401  (import keeps bass registered)
import concourse.mybir as mybir
import concourse.tile as tile
from concourse import bacc
from concourse.bass_utils import run_bass_kernel_spmd
from concourse.masks import make_identity

N_CORES = 8
B = 16384
BSH = B // N_CORES          # 2048 batch rows per core
DIN = 512
K = 8                       # cascade stages
WS = 256                    # neurons per stage
DOUT = 128
EPS = 1e-5
P = 128
NB = BSH // 512             # batch chunks of 512 (PSUM bank free dim)
KO = [(DIN + WS * i) // P for i in range(K)]   # k-tiles per stage: 4,6,...,18
T_TOT = (DIN + WS * K) // P                    # 20 F tiles
HACC_STAGES = (2, 3, 4, 5, 6, 7)  # stages whose x-part is pre-accumulated

_NC_CACHE = {}

# test-harness knobs (ignored in normal use): when TRACE_DIR is set the
# device run is profiled and kernel() stores the BassKernelResults here.
TRACE_DIR = None
LAST_RESULTS = None

BF = mybir.dt.bfloat16
F32 = mybir.dt.float32


def _build_nc():
    nc = bacc.Bacc("TRN2", target_bir_lowering=False, debug=False,
                   num_devices=N_CORES)

    xt_d = nc.dram_tensor("xt", [P, DIN // P, BSH], BF, kind="ExternalInput")
    w_d = [
        nc.dram_tensor(f"w{i}", [P, KO[i], WS], BF, kind="ExternalInput")
        for i in range(K)
    ]
    wo_d = nc.dram_tensor("wo", [P, T_TOT, DOUT], BF, kind="ExternalInput")
    bv_d = nc.dram_tensor("bv", [P, K, 2], F32, kind="ExternalInput")
    gv_d = nc.dram_tensor("gv", [P, K, 2], F32, kind="ExternalInput")
    bev_d = nc.dram_tensor("bev", [P, K, 2], F32, kind="ExternalInput")
    bout_d = nc.dram_tensor("boutv", [P, 1], F32, kind="ExternalInput")
    outT_d = nc.dram_tensor("outT", [P, BSH], F32, kind="ExternalOutput")

    with tile.TileContext(nc) as tc:
        _emit(nc, tc, xt_d, w_d, wo_d, bv_d, gv_d, bev_d, bout_d, outT_d)
    nc.compile()
    return nc


def _emit(nc, tc, xt_d, w_d, wo_d, bv_d, gv_d, bev_d, bout_d, outT_d):
    AF = mybir.ActivationFunctionType
    OP = mybir.AluOpType
    groups = [list(range(N_CORES))]

    with (
        tc.tile_pool(name="big", bufs=1) as big,
        tc.tile_pool(name="hp", bufs=1) as hp,
        tc.tile_pool(name="scrp", bufs=3) as scrp,
        tc.tile_pool(name="small", bufs=2) as small,
        tc.tile_pool(name="ps", bufs=8, space="PSUM") as ps,
        tc.tile_pool(name="dram", bufs=2, space="DRAM") as dram,
    ):
        # ---- persistent SBUF ----
        F = [big.tile([P, BSH], BF, tag=f"F{t}", name=f"F{t}") for t in range(T_TOT)]
        Wsb = [big.tile([P, KO[i], WS], BF, tag=f"W{i}", name=f"W{i}") for i in range(K)]
        WO = big.tile([P, T_TOT, DOUT], BF, tag="WO")
        BV = big.tile([P, K, 2], F32, tag="BV")
        GV = big.tile([P, K, 2], F32, tag="GV")
        BEV = big.tile([P, K, 2], F32, tag="BEV")
        BOUT = big.tile([P, 1], F32, tag="BOUT")
        OUTACC = big.tile([P, BSH], F32, tag="OUTACC")
        EPSC = big.tile([P, 1], F32, tag="EPSC")
        nc.vector.memset(EPSC[:], EPS)
        # identity (bf16) for re-injecting SBUF partial sums into PSUM
        IDT = big.tile([P, P], BF, tag="IDT")
        make_identity(nc, IDT[:, :])
        # x-block partial pre-accumulation targets for stages 2..5: their
        # x-part matmuls run inside the long first-collective window and
        # are re-injected later via an (exact) identity matmul.
        HACC = {j: big.tile([P, 2, BSH], BF, tag=f"HACC{j}", name=f"HACC{j}")
                for j in HACC_STAGES}

        # ---- warm-up collective: absorbs the one-time barrier/bootstrap
        # cost (~65us observed) so the first real AllGather runs at the
        # steady-state ~5us latency. Triggered at t=0, result unused.
        warm_in = dram.tile([P, 2], F32, tag="warm_in")
        warm_out = dram.tile([N_CORES, P, 2], F32, tag="warm_out",
                             addr_space="Shared")
        nc.gpsimd.collective_compute(
            "AllGather", OP.bypass, replica_groups=groups,
            ins=[warm_in.opt()], outs=[warm_out.opt()],
        )

        # ---- input DMAs: three queues, ordered by first consumption ----
        # sync: x-blocks then late-stage weights; gpsimd: stage-0 weights,
        # output weights, small vectors; scalar: early-stage weights.
        for ko in range(KO[0]):
            nc.gpsimd.dma_start(Wsb[0][:, ko, :], w_d[0][:, ko, :])
        nc.gpsimd.dma_start(WO[:], wo_d[:, :, :])
        nc.gpsimd.dma_start(BV[:], bv_d[:, :, :])
        nc.gpsimd.dma_start(GV[:], gv_d[:, :, :])
        nc.gpsimd.dma_start(BEV[:], bev_d[:, :, :])
        nc.gpsimd.dma_start(BOUT[:], bout_d[:, :])
        nc.gpsimd.dma_start(Wsb[3][:], w_d[3][:, :, :])
        nc.gpsimd.dma_start(Wsb[7][:], w_d[7][:, :, :])
        for t in range(DIN // P):
            eng = nc.sync if t % 2 == 0 else nc.scalar
            for bb in range(NB):
                eng.dma_start(F[t][:, bb * 512:(bb + 1) * 512],
                              xt_d[:, t, bb * 512:(bb + 1) * 512])
        nc.sync.dma_start(Wsb[5][:], w_d[5][:, :, :])
        nc.sync.dma_start(Wsb[6][:], w_d[6][:, :, :])
        nc.scalar.dma_start(Wsb[1][:], w_d[1][:, :, :])
        nc.scalar.dma_start(Wsb[2][:], w_d[2][:, :, :])
        nc.scalar.dma_start(Wsb[4][:], w_d[4][:, :, :])

        # stage j's k-tiles pre-accumulated into HACC[j] before its real
        # PSUM group materializes (parked in SBUF, re-injected exactly via
        # an identity matmul)
        PREACC = {2: 4, 3: 4, 4: 6, 5: 8, 6: 6, 7: 8}  # parked k-tiles

        def mm(pt, lhsT, k, bb, start, stop):
            nc.tensor.matmul(
                pt[:, :], lhsT, F[k][:, bb * 512:(bb + 1) * 512],
                start=start, stop=stop)

        def alloc_group():
            return [
                [ps.tile([P, 512], F32, tag="pt", name="pt") for _ in range(NB)]
                for _ in range(2)
            ]

        def hacc_group(j, ks, init):
            """Advance stage j's parked partial sum by k-tiles ``ks``."""
            pa = alloc_group()
            for n in range(2):
                for bb in range(NB):
                    if not init:
                        nc.tensor.matmul(
                            pa[n][bb][:, :], IDT[:, :],
                            HACC[j][:, n, bb * 512:(bb + 1) * 512],
                            start=True, stop=False)
                for k in ks:
                    lhsT = Wsb[j][:, k, n * P:(n + 1) * P]
                    for bb in range(NB):
                        mm(pa[n][bb], lhsT, k, bb,
                           init and k == ks[0], k == ks[-1])
            for n in range(2):
                for bb in range(NB):
                    nc.vector.tensor_copy(
                        HACC[j][:, n, bb * 512:(bb + 1) * 512], pa[n][bb][:, :])

        def real_early(j, psums):
            """Ungated part of stage j's real contraction."""
            pre = PREACC.get(j, 0)
            for n in range(2):
                if pre:
                    for bb in range(NB):
                        nc.tensor.matmul(
                            psums[n][bb][:, :], IDT[:, :],
                            HACC[j][:, n, bb * 512:(bb + 1) * 512],
                            start=True, stop=False)
                for k in range(pre, KO[j] - 2):
                    lhsT = Wsb[j][:, k, n * P:(n + 1) * P]
                    for bb in range(NB):
                        mm(psums[n][bb], lhsT, k, bb,
                           pre == 0 and k == 0, False)

        def real_late(j, psums):
            # chunk-major order: psum (n, bb) groups complete progressively
            # so the relu/stats pipeline starts before the last matmul
            for bb in range(NB):
                for n in range(2):
                    for k in (KO[j] - 2, KO[j] - 1):
                        lhsT = Wsb[j][:, k, n * P:(n + 1) * P]
                        mm(psums[n][bb], lhsT, k, bb, False, k == KO[j] - 1)

        def out_group(ks, first):
            pso = [ps.tile([P, 512], F32, tag="pt", name="pt")
                   for _ in range(NB)]
            for k in ks:
                lhsT = WO[:, k, :]
                for bb in range(NB):
                    nc.tensor.matmul(
                        pso[bb][:, :], lhsT,
                        F[k][:, bb * 512:(bb + 1) * 512],
                        start=(k == ks[0]), stop=(k == ks[-1]))
            for bb in range(NB):
                dst = OUTACC[:, bb * 512:(bb + 1) * 512]
                if first:
                    nc.vector.tensor_scalar_add(dst, pso[bb][:, :],
                                                BOUT[:, 0:1])
                else:
                    nc.vector.tensor_add(dst, dst, pso[bb][:, :])

        # per-window AR-independent fill: list of thunks emitted between
        # the AllGather trigger and the BN-coefficient computation of each
        # iteration, sized to cover the collective's latency
        WINDOW_FILL = {
            0: [lambda: out_group(range(4), True),
                lambda: hacc_group(2, range(4), True),
                lambda: hacc_group(3, range(4), True),
                lambda: hacc_group(4, range(4), True),
                lambda: hacc_group(5, range(4), True),
                lambda: hacc_group(7, range(4), True),
                lambda: hacc_group(6, range(4), True)],
            1: [lambda: hacc_group(4, (4, 5), False),
                lambda: hacc_group(5, (4, 5), False)],
            2: [lambda: hacc_group(5, (6, 7), False),
                lambda: hacc_group(6, (4, 5), False)],
            3: [lambda: hacc_group(7, (4, 5), False)],
            4: [lambda: hacc_group(7, (6, 7), False)],
            5: [],
            6: [],
            7: [lambda: out_group(range(4, T_TOT - 2), False)],
        }

        # stage 0: everything available immediately
        psums = alloc_group()
        real_early(0, psums)
        real_late(0, psums)

        for i in range(K):
            # ---- relu + bias: PSUM -> bf16 h in SBUF ----
            # n=0 chunks on ScalarE, n=1 chunks on VectorE so both n-tiles
            # clear PSUM (and feed bn_stats) in parallel.
            hs = [hp.tile([P, BSH], BF, tag=f"h{n}", name=f"h{n}") for n in range(2)]
            sums = small.tile([P, NB], F32, tag="sums")
            sqs = small.tile([P, NB], F32, tag="sqs")
            st = small.tile([P, NB, 6], F32, tag="st")
            mv1 = small.tile([P, 2], F32, tag="mv1")
            arin = small.tile([P, 2, 2], F32, tag="arin")
            # stats payload = raw (sum, sum-of-squares) pairs, additive
            # across cores. n=0 on ScalarE (relu/square with accum_out),
            # n=1 on VectorE (relu + bn_stats), in psum-completion order so
            # both pipelines drain right behind the last matmul.
            for bb in range(NB):
                c0 = hs[0][:, bb * 512:(bb + 1) * 512]
                c1 = hs[1][:, bb * 512:(bb + 1) * 512]
                nc.scalar.activation(
                    c0, psums[0][bb][:, :], AF.Relu,
                    bias=BV[:, i, 0:1], scale=1.0,
                    accum_out=sums[:, bb:bb + 1],
                )
                scr = scrp.tile([P, 512], BF, tag="scr", name="scr")
                nc.scalar.activation(
                    scr[:, :], c0, AF.Square,
                    accum_out=sqs[:, bb:bb + 1])
                nc.vector.tensor_scalar(
                    c1, psums[1][bb][:, :], BV[:, i, 1:2], 0.0,
                    op0=OP.add, op1=OP.max,
                )
                nc.vector.bn_stats(st[:, bb, :], c1)
            nc.vector.bn_aggr(mv1[:], st[:, :, :])
            # n=1: (mean, var) -> raw sums;  n=0: reduce the chunk sums
            nc.vector.tensor_scalar(
                arin[:, 1, 1:2], mv1[:, 0:1], mv1[:, 0:1], mv1[:, 1:2],
                op0=OP.mult, op1=OP.add)
            nc.vector.tensor_scalar_mul(arin[:, 1, 1:2], arin[:, 1, 1:2],
                                        float(BSH))
            nc.vector.tensor_scalar_mul(arin[:, 1, 0:1], mv1[:, 0:1],
                                        float(BSH))
            nc.vector.tensor_reduce(
                arin[:, 0, 0:1], sums[:, :], axis=mybir.AxisListType.X,
                op=OP.add)
            nc.vector.tensor_reduce(
                arin[:, 0, 1:2], sqs[:, :], axis=mybir.AxisListType.X,
                op=OP.add)

            # ---- cross-core exchange of the stats (2KB): AllGather has a
            # ~2x lower latency floor than AllReduce; the 8-way sum is done
            # locally on VectorE ----
            ccin = dram.tile([P, 2, 2], F32, tag="ccin")
            ccout = dram.tile([N_CORES, P, 2, 2], F32, tag="ccout",
                              addr_space="Shared")
            nc.sync.dma_start(ccin[:], arin[:])
            nc.gpsimd.collective_compute(
                "AllGather", OP.bypass, replica_groups=groups,
                ins=[ccin.opt()], outs=[ccout.opt()],
            )
            ag = small.tile([P, N_CORES, 2, 2], F32, tag="ag")
            nc.sync.dma_start(ag[:], ccout[:, :, :, :].rearrange(
                "r p a b -> p r a b"))
            ared = small.tile([P, 2, 2], F32, tag="ared")
            nc.vector.tensor_reduce(
                ared[:, :, :], ag[:, :, :, :].rearrange("p r a b -> p a b r"),
                axis=mybir.AxisListType.X, op=OP.add)

            # ---- overlap window: AR-independent matmul fill ----
            for thunk in WINDOW_FILL[i]:
                thunk()
            if i < K - 1:
                psums = alloc_group()
                real_early(i + 1, psums)

            # ---- BN affine coefficients from global stats ----
            mue = small.tile([P, 2, 2], F32, tag="mue")
            nv = small.tile([P, 2], F32, tag="nv")
            rstd = small.tile([P, 2], F32, tag="rstd")
            a_ = small.tile([P, 2], F32, tag="a_")
            negc = small.tile([P, 2], F32, tag="negc")
            nc.vector.tensor_scalar_mul(mue[:], ared[:, :, :], 1.0 / B)
            mu = mue[:, :, 0]
            nc.vector.tensor_mul(nv[:], mu, mu)
            nc.vector.tensor_sub(nv[:], nv[:], mue[:, :, 1])   # mu^2 - E2 = -var
            nc.scalar.activation(rstd[:], nv[:], AF.Sqrt,
                                 bias=EPSC[:, 0:1], scale=-1.0)  # sqrt(var+eps)
            nc.vector.reciprocal(rstd[:], rstd[:])
            nc.vector.tensor_mul(a_[:], GV[:, i, :], rstd[:])
            nc.vector.tensor_mul(negc[:], mu, a_[:])
            nc.vector.tensor_sub(negc[:], negc[:], BEV[:, i, :])  # a*mu - beta

            # ---- normalize into the F blocks (bf16), chunked so the
            # first gated matmul can start before the whole tile is done ----
            for n in range(2):
                for q in range(NB):
                    sl = slice(q * 512, (q + 1) * 512)
                    nc.vector.tensor_scalar(
                        F[DIN // P + 2 * i + n][:, sl], hs[n][:, sl],
                        a_[:, n:n + 1], negc[:, n:n + 1],
                        op0=OP.mult, op1=OP.subtract,
                    )

            # ---- gated (late) matmuls of the next stage ----
            if i < K - 1:
                real_late(i + 1, psums)

        # ---- epilogue: last two output k-tiles + store ----
        out_group((T_TOT - 2, T_TOT - 1), False)
        for bb in range(NB):
            nc.sync.dma_start(outT_d[:, bb * 512:(bb + 1) * 512],
                              OUTACC[:, bb * 512:(bb + 1) * 512])


def _get_nc():
    if "nc" not in _NC_CACHE:
        _NC_CACHE["nc"] = _build_nc()
    return _NC_CACHE["nc"]


def kernel(x, W0, W1, W2, W3, W4, W5, W6, W7, b, gamma, beta, Wout, bout):
    Ws = [W0, W1, W2, W3, W4, W5, W6, W7]
    nc = _get_nc()

    def pack_vec(v):  # [8,256] -> [128, 8, 2]
        return np.ascontiguousarray(
            np.asarray(v, np.float32).reshape(K, 2, P).transpose(2, 0, 1))

    common = {}
    for i, W in enumerate(Ws):
        wt = np.asarray(W, np.float32).T.astype(bfloat16)        # [d_i, 256]
        common[f"w{i}"] = np.ascontiguousarray(
            wt.reshape(KO[i], P, WS).transpose(1, 0, 2))         # [128, ko, 256]
    wot = np.asarray(Wout, np.float32).T.astype(bfloat16)        # [2560, 128]
    common["wo"] = np.ascontiguousarray(
        wot.reshape(T_TOT, P, DOUT).transpose(1, 0, 2))          # [128, 20, 128]
    common["bv"] = pack_vec(b)
    common["gv"] = pack_vec(gamma)
    common["bev"] = pack_vec(beta)
    common["boutv"] = np.ascontiguousarray(
        np.asarray(bout, np.float32).reshape(P, 1))

    in_maps = []
    for c in range(N_CORES):
        xs = np.asarray(x[c * BSH:(c + 1) * BSH], np.float32)    # [2048, 512]
        xt = xs.T.astype(bfloat16)                               # [512, 2048]
        in_maps.append({
            **common,
            "xt": np.ascontiguousarray(
                xt.reshape(DIN // P, P, BSH).transpose(1, 0, 2)),
        })

    kw = {}
    if TRACE_DIR is not None:
        kw = dict(trace=True, tmpdir=TRACE_DIR)
    try:
        res = run_bass_kernel_spmd(nc, in_maps, list(range(N_CORES)), **kw)
    except Exception:
        # transient PJRT INTERNAL errors have been observed; retry once
        res = run_bass_kernel_spmd(nc, in_maps, list(range(N_CORES)), **kw)
    global LAST_RESULTS
    LAST_RESULTS = res
    out = np.empty((B, DOUT), np.float32)
    for c in range(N_CORES):
        out[c * BSH:(c + 1) * BSH] = res.results[c]["outT"].T
    return out

